# revision 1
# baseline (speedup 1.0000x reference)
"""MQA attention kernel for Trainium2 (8 NeuronCores, Bass/Tile).

Problem: Q [2,16,2048,64], K/V [2,1,2048,64] fp32, out = softmax(QK^T/8) V.

Sharding: 32 (batch, head) pairs over 8 cores -> 4 heads per core; each core
gets one batch's K/V (replicated across the 4 cores of that batch).

Per-core algorithm (S^T orientation so softmax reduction lands on the free dim
and PV needs no transposition of P):
  - K^T, Q^T built on-chip via PE transposes (d=64 on partitions, zero-padded
    to 128 so all matmuls contract over the full partition dim).
  - S^T[j, q] = (K Q^T) computed in fp32r matmuls (full-rate 4-byte dtype),
    PSUM bank per 128-row j-chunk.
  - exp(s/8) fused with PSUM->SBUF evacuation on the scalar engine (no max
    subtraction: scores/8 ~ N(0,1), exp never overflows fp32).
  - PV uses V augmented with a ones column: one matmul chain yields both
    O^T = V'^T P^T and the softmax denominators (row 64).
  - O'^T transposed back with PE, normalized with DVE reciprocal+mul, DMA out.

The q-rows are processed in an interleaved order (partition p holds rows
16p+c) so every DMA moves contiguous 4KB runs; the same rearrange on the
output store undoes the permutation.

Scheduling: K^T and head-0 Q^T transposes are interleaved right after the
input DMAs (first exp starts ~8us in); head h+1's Q^T transposes are hoisted
into head h's first q-block so the scalar engine never stalls at head
boundaries. PSUM budget (8 banks): 2x2 rotating score banks + 1 PV
accumulator + 2 transpose staging.

Measured on trn2 (NTFF profile, core 0): 180.4 us end-to-end, scalar engine
(exp, the only transcendental unit) ~80% busy = the roofline driver;
rel err vs fp64-ish jax reference: 1.9e-4 (fp32r matmuls).
"""

import numpy as np

import concourse.bass as bass
import concourse.mybir as mybir
import concourse.tile as tile
from concourse import bacc
from concourse.bass_utils import run_bass_kernel_spmd
from concourse.masks import make_identity

B, H, S, D = 2, 16, 2048, 64
N_CORES = 8
HPC = (B * H) // N_CORES  # heads per core = 4
P = 128
NJ = S // P               # 16 key chunks of 128
QB = 512                  # queries per block (= max fp32 matmul free dim)
NQB = S // QB             # 4 q-blocks per head
SCALE = 1.0 / float(D) ** 0.5
F32 = mybir.dt.float32
F32R = mybir.dt.float32r  # 4-byte matmul dtype, 2 cyc/row streaming
BF16 = mybir.dt.bfloat16
EXP_GRP = 2               # j-chunks (PSUM banks) per exp ACTIVATE group
ROW_TILE = False          # run QK^T matmul pairs in PE row-groups 0-63 / 64-127

_CACHED = {}
# Best measured config (profiled on HW); kernel() uses this.
DEFAULT_CFG = {}


def _build_module(reps=1, **cfg):
    nc = bacc.Bacc(None)
    q = nc.dram_tensor("q", [HPC, S, D], F32, kind="ExternalInput")
    k = nc.dram_tensor("k", [S, D], F32, kind="ExternalInput")
    v = nc.dram_tensor("v", [S, D], F32, kind="ExternalInput")
    o = nc.dram_tensor("o", [HPC, S, D], F32, kind="ExternalOutput")

    with tile.TileContext(nc) as tc:
        with tc.tile_pool(name="const", bufs=1) as cpool:
            identity = cpool.tile([P, P], F32)
            make_identity(nc, identity)

            kT = cpool.tile([P, S], F32R)
            nc.gpsimd.memset(kT[64:P, :].bitcast(mybir.dt.uint32), 0)
            vp = cpool.tile([P, NJ, D + 1], F32R)
            nc.gpsimd.memset(vp[:, :, D].bitcast(mybir.dt.uint32), 0x3F800000)
            qT_tiles = []
            for i in range(2):
                qTt = cpool.tile([P, S], F32R, name=f"qT{i}")
                nc.gpsimd.memset(qTt[64:P, :].bitcast(mybir.dt.uint32), 0)
                qT_tiles.append(qTt)

            # `reps` re-traces the whole per-core program body (for
            # differential wall-clock timing of the device work without RPC
            # dispatch overhead); the functional kernel uses reps=1.
            for rep in range(reps):
                _trace_body(nc, tc, q, k, v, o, identity, kT, vp, qT_tiles, **cfg)
    nc.compile()
    return nc


def _trace_body(
    nc, tc, q, k, v, o, identity, kT, vp, qT_tiles,
    exp_grp=None, row_tile=None, pt_bufs=2, sg_bufs=2, tr_bufs=2, pv_bf16=False,
):
    EXP_GRP = exp_grp if exp_grp is not None else globals()["EXP_GRP"]
    ROW_TILE = row_tile if row_tile is not None else globals()["ROW_TILE"]
    with (
        tc.tile_pool(name="natb", bufs=2) as npool,
        tc.tile_pool(name="workb", bufs=pt_bufs) as wpool,
        tc.tile_pool(name="psb", bufs=sg_bufs, space="PSUM") as pspool,
        tc.tile_pool(name="ps1b", bufs=1, space="PSUM") as ps1pool,
    ):
            def transpose_64(dst, src_nat, who):
                # PE-transpose 4 [128,64] chunks into one PSUM tile, then one
                # DVE cast into [64, 512] of the fp32r destination.
                for g in range(NJ // 4):
                    pst = ps1pool.tile(
                        [64, 4, P], F32, tag="tr", bufs=tr_bufs, name=f"pst_{who}{g}"
                    )
                    for t in range(4):
                        nc.tensor.transpose(
                            pst[:, t, :], src_nat[:, 4 * g + t, :], identity
                        )
                    nc.vector.tensor_copy(dst[0:64, 512 * g : 512 * (g + 1)], pst[:])

            def load_q(h):
                q_nat = npool.tile([P, NJ, D], F32, tag="nat", name=f"q_nat{h}")
                nc.sync.dma_start(q_nat[:], q[h].rearrange("(p c) d -> p c d", p=P))
                return q_nat

            def prep_qT(h, q_nat):
                qT = qT_tiles[h % 2]
                transpose_64(qT, q_nat, f"q{h}_")
                if ROW_TILE:
                    nc.sync.dma_start(qT[64:P, :], qT[0:64, :])
                return qT

            # ---- startup: K^T and head-0 Q^T, transposed interleaved ----
            k_nat = npool.tile([P, NJ, D], F32, tag="nat")
            nc.sync.dma_start(k_nat[:], k.rearrange("(p c) d -> p c d", p=P))
            q_nat_next = load_q(0)
            for g in range(NJ // 4):
                pstk = ps1pool.tile([64, 4, P], F32, tag="tr", bufs=tr_bufs, name=f"pst_k{g}")
                for t in range(4):
                    nc.tensor.transpose(pstk[:, t, :], k_nat[:, 4 * g + t, :], identity)
                nc.vector.tensor_copy(kT[0:64, 512 * g : 512 * (g + 1)], pstk[:])
                pstq = ps1pool.tile([64, 4, P], F32, tag="tr", bufs=tr_bufs, name=f"pst_q0{g}")
                for t in range(4):
                    nc.tensor.transpose(
                        pstq[:, t, :], q_nat_next[:, 4 * g + t, :], identity
                    )
                nc.vector.tensor_copy(
                    qT_tiles[0][0:64, 512 * g : 512 * (g + 1)], pstq[:]
                )
            if ROW_TILE:
                nc.sync.dma_start(kT[64:P, :], kT[0:64, :])
                nc.sync.dma_start(qT_tiles[0][64:P, :], qT_tiles[0][0:64, :])

            # ---- V' [128, 16, 65]: V plus a ones column (softmax denom) ----
            v_nat = npool.tile([P, NJ, D], F32, tag="nat", name="v_nat")
            nc.sync.dma_start(v_nat[:], v.rearrange("(p c) d -> p c d", p=P))
            if pv_bf16:
                vpb = wpool.tile([P, NJ, D + 1], BF16, tag="vpb", bufs=1, name="vpb")
                nc.gpsimd.memset(vpb[:, :, D], 1.0)
                nc.vector.tensor_copy(vpb[:, :, 0:D], v_nat[:])
                vp = vpb
            else:
                nc.vector.tensor_copy(vp[:, :, 0:D], v_nat[:])

            for h in range(HPC):
                qT = qT_tiles[h % 2]

                for qb in range(NQB):
                    qs = qT[:, QB * qb : QB * (qb + 1)]
                    # exp(S^T/8): j-chunk scores into PSUM, scalar engine
                    # evacuates each EXP_GRP-bank group with a fused exp.
                    pT = wpool.tile([P, NJ * QB], BF16 if pv_bf16 else F32R, tag="pT", name=f"pT{h}_{qb}")
                    if EXP_GRP == 3:
                        group_sizes = [3, 3, 3, 3, 2, 2]
                    else:
                        group_sizes = [EXP_GRP] * (NJ // EXP_GRP)
                    g_start = [sum(group_sizes[:i]) for i in range(len(group_sizes))]
                    for g, gsz in enumerate(group_sizes):
                        sg = pspool.tile(
                            [P, gsz, QB],
                            F32,
                            tag="sg",
                            name=f"sg{h}_{qb}_{g}",
                            padded_shape=[P, max(group_sizes), QB],
                        )
                        for i in range(gsz):
                            j = g_start[g] + i
                            if ROW_TILE:
                                half = slice(0, 64) if i % 2 == 0 else slice(64, P)
                                nc.tensor.matmul(
                                    sg[:, i, :],
                                    lhsT=kT[half, P * j : P * (j + 1)],
                                    rhs=qs[half, :],
                                    start=True,
                                    stop=True,
                                )
                            else:
                                nc.tensor.matmul(
                                    sg[:, i, :],
                                    lhsT=kT[:, P * j : P * (j + 1)],
                                    rhs=qs,
                                    start=True,
                                    stop=True,
                                )
                        nc.scalar.activation(
                            pT[:, QB * g_start[g] : QB * (g_start[g] + gsz)],
                            sg[:],
                            mybir.ActivationFunctionType.Exp,
                            scale=SCALE,
                        )
                    # O'^T [65, 512] = V'^T P^T accumulated over j-chunks
                    pv = ps1pool.tile([D + 1, QB], F32, tag="pv", name=f"pv{h}_{qb}")
                    for c in range(NJ):
                        nc.tensor.matmul(
                            pv[:],
                            lhsT=vp[:, c, :],
                            rhs=pT[:, QB * c : QB * (c + 1)],
                            start=(c == 0),
                            stop=(c == NJ - 1),
                        )
                    oev = wpool.tile([D + 1, QB], F32, tag="oev", name=f"oev{h}_{qb}")
                    nc.vector.tensor_copy(oev[:], pv[:])
                    # transpose back to [q, d], normalize rows by the denom
                    otr = ps1pool.tile(
                        [P, 4, D + 1], F32, tag="tr", bufs=tr_bufs, name=f"otr{h}_{qb}"
                    )
                    rcp = wpool.tile([P, 4], F32, tag="rcp", name=f"rcp{h}_{qb}")
                    oout = wpool.tile([P, 4, D], F32, tag="oout", name=f"oout{h}_{qb}")
                    for t in range(4):
                        nc.tensor.transpose(
                            otr[:, t, :],
                            oev[:, P * t : P * (t + 1)],
                            identity[0 : D + 1, 0 : D + 1],
                        )
                        nc.vector.reciprocal(rcp[:, t : t + 1], otr[:, t, D : D + 1])
                        nc.vector.tensor_scalar(
                            oout[:, t, :],
                            otr[:, t, 0:D],
                            rcp[:, t : t + 1],
                            None,
                            mybir.AluOpType.mult,
                        )
                    nc.sync.dma_start(
                        o[h].rearrange("(p c) d -> p c d", p=P)[
                            :, 4 * qb : 4 * (qb + 1), :
                        ],
                        oout[:],
                    )
                    if qb == 0 and h + 1 < HPC:
                        q_nat_next = load_q(h + 1)
                        prep_qT(h + 1, q_nat_next)
    nc.compile()
    return nc


def _get_module(reps=1, **cfg):
    key = (reps, tuple(sorted(cfg.items())))
    if key not in _CACHED:
        _CACHED[key] = _build_module(reps, **cfg)
    return _CACHED[key]


def make_in_maps(Q, K, V):
    """Shard full inputs into per-core input maps (core c -> batch c//4,
    heads 4*(c%4)..4*(c%4)+4)."""
    Q = np.asarray(Q, dtype=np.float32)
    K = np.asarray(K, dtype=np.float32)
    V = np.asarray(V, dtype=np.float32)
    in_maps = []
    for c in range(N_CORES):
        b = c // (N_CORES // B)
        h0 = HPC * (c % (N_CORES // B))
        in_maps.append(
            {
                "q": np.ascontiguousarray(Q[b, h0 : h0 + HPC]),
                "k": np.ascontiguousarray(K[b, 0]),
                "v": np.ascontiguousarray(V[b, 0]),
            }
        )
    return in_maps


def assemble_output(results):
    out = np.empty((B, H, S, D), dtype=np.float32)
    for c in range(N_CORES):
        b = c // (N_CORES // B)
        h0 = HPC * (c % (N_CORES // B))
        out[b, h0 : h0 + HPC] = results[c]["o"]
    return out


def kernel(Q, K, V):
    nc = _get_module(1, **DEFAULT_CFG)
    res = run_bass_kernel_spmd(nc, make_in_maps(Q, K, V), core_ids=list(range(N_CORES)))
    return assemble_output(res.results)



# revision 4
# speedup vs baseline: 1.2737x; 1.2737x over previous
"""MQA attention kernel v3 for Trainium2 (8 NeuronCores, Bass/Tile).

Problem: Q [2,16,2048,64], K/V [2,1,2048,64] fp32, out = softmax(QK^T/8) V.
Sharding: 32 (batch, head) pairs over 8 cores -> 4 heads/core; one batch's
K/V per core.

v3 = v2's data path with a software-pipelined emission order:
  - bf16 everywhere; Q^T/K^T via gpsimd DMA-cast + xbar DMA transposes of
    [128,128] chunk-pairs (zero PE/DVE cost). kTs + half-swapped kTs2 give
    every chunk in both partition halves.
  - Unit = (head, 512-col q-block): 16 score steps per unit; each step is
    one [64-deep] QK matmul pair (parities in the two PE row halves), one
    exp instruction (ScalarE table-exp for 12 steps, VectorE Schraudolph
    int16->bf16 for 4), plus TWO PV-chain matmuls of the PREVIOUS unit
    threaded between - so ScalarE never idles at unit boundaries.
  - PV: full-128-deep accumulation chains (one PSUM bank per col-group,
    LDWEIGHTS hides in the background weight buffer).
  - Output: PE transpose -> DVE PSUM->SBUF copy -> gpsimd normalize_recip
    (the only engine with slack) -> DMA store.

The q columns are processed in a (parity, block) interleaved order; the
output store APs undo the permutation (col-group (b, parity p, sub t)
holds global q with q mod 16 == 8b + 2t + p).
"""

import numpy as np

import concourse.bass as bass
import concourse.mybir as mybir
import concourse.tile as tile
from concourse import bacc
from concourse.bass_utils import run_bass_kernel_spmd
from concourse.masks import make_identity

B, H, S, D = 2, 16, 2048, 64
N_CORES = 8
HPC = (B * H) // N_CORES
P = 128
NJ = S // P
NG = NJ // 2
QB = 512
LAG = 4
SCALE = 1.0 / float(D) ** 0.5
F32 = mybir.dt.float32
BF16 = mybir.dt.bfloat16
I16 = mybir.dt.int16

LOG2E = 1.4426950408889634
A16 = float((1 << 7) * LOG2E * SCALE)
B16 = float(127.0 * (1 << 7) - 7.42)

# exp steps (tt, jj) run on the DVE via Schraudolph; listing both tt of a
# jj approximates chunks {2jj, 2jj+1} fully.
DVE_TILES = ((0, 1), (0, 3), (0, 5), (1, 1), (1, 3))

_CACHED = {}
DEFAULT_CFG = {}


def _build_module(**cfg):
    nc = bacc.Bacc(None)
    q = nc.dram_tensor("q", [HPC, S, D], F32, kind="ExternalInput")
    k = nc.dram_tensor("k", [S, D], F32, kind="ExternalInput")
    v = nc.dram_tensor("v", [S, D], F32, kind="ExternalInput")
    o = nc.dram_tensor("o", [HPC, S, D], F32, kind="ExternalOutput")
    _trace_body(nc, q, k, v, o, **cfg)
    nc.compile()
    return nc


def _trace_body(nc, q, k, v, o, sg_bufs=3, pv_bufs=2, pt_bufs=3, dve_tiles=None):
    dve_tiles = DVE_TILES if dve_tiles is None else dve_tiles
    with tile.TileContext(nc) as tc:
        with (
            tc.tile_pool(name="const", bufs=1) as cpool,
            tc.tile_pool(name="qb", bufs=2) as qpool,
            tc.tile_pool(name="ptb", bufs=pt_bufs) as ptpool,
            tc.tile_pool(name="wk", bufs=2) as wpool,
            tc.tile_pool(name="sg", bufs=sg_bufs, space="PSUM") as sgpool,
            tc.tile_pool(name="pv", bufs=pv_bufs, space="PSUM") as pvpool,
        ):
            k_bf = cpool.tile([P, NJ, D], BF16)
            k_src = k.rearrange("(p c) d -> p c d", p=P)
            nc.gpsimd.dma_start(k_bf[:, 0 : NJ // 2, :], k_src[:, 0 : NJ // 2, :])
            nc.gpsimd.dma_start(k_bf[:, NJ // 2 :, :], k_src[:, NJ // 2 :, :])
            kTs = cpool.tile([P, NG, P], BF16)
            kTs2 = cpool.tile([P, NG, P], BF16)
            for g in range(NG):
                nc.scalar.dma_start_transpose(
                    kTs[:, g, :], k_bf[:, 2 * g : 2 * g + 2, :]
                )
            nc.sync.dma_start(kTs2[0:64, :, :], kTs[64:P, :, :])
            nc.sync.dma_start(kTs2[64:P, :, :], kTs[0:64, :, :])

            identity = cpool.tile([P, P], F32)
            make_identity(nc, identity)

            v_nat = cpool.tile([P, NJ, D], F32)
            nc.sync.dma_start(v_nat[:], v.rearrange("(p c) d -> p c d", p=P))
            vp = cpool.tile([P, NJ, D + 1], BF16)
            nc.gpsimd.memset(vp[:, :, D], 1.0)
            nc.vector.tensor_copy(vp[:, :, 0:D], v_nat[:])

            def load_qT(h):
                q_bf = qpool.tile([P, NJ, D], BF16, tag="qbf", name=f"qbf{h}")
                q_src = q[h].rearrange("(p c) d -> p c d", p=P)
                nc.gpsimd.dma_start(q_bf[:, 0 : NJ // 2, :], q_src[:, 0 : NJ // 2, :])
                nc.gpsimd.dma_start(q_bf[:, NJ // 2 :, :], q_src[:, NJ // 2 :, :])
                qTs = qpool.tile([P, NG, P], BF16, tag="qTs", name=f"qTs{h}")
                for g in range(NG):
                    nc.sync.dma_start_transpose(
                        qTs[:, g, :], q_bf[:, 2 * g : 2 * g + 2, :]
                    )
                return qTs

            def pt_idx(p, c):
                jj, jp = divmod(c, 2)
                if p == 0:
                    return (0, jj, 0) if jp == 0 else (1, jj, 0)
                return (0, jj, 1) if jp == 1 else (1, jj, 1)

            qTs_cur = load_qT(0)

            # ---- steady-state pipeline over units u = (h, b); PV of unit
            # u-1 threads through unit u's score steps ----
            units = [(h, b) for h in range(HPC) for b in range(2)]
            qTs_of = {0: qTs_cur}
            pT_of = {}
            pv_of = {}

            def pt_idx(p, c):
                jj, jp = divmod(c, 2)
                if p == 0:
                    return (0, jj, 0) if jp == 0 else (1, jj, 0)
                return (0, jj, 1) if jp == 1 else (1, jj, 1)

            def emit_output(h, b, p, pv):
                otr = pvpool.tile(
                    [P, 4, D + 1], F32, tag="pv", name=f"otr{h}_{b}_{p}",
                    padded_shape=[P, 4, QB // 4],
                )
                oev = wpool.tile([D + 1, QB], F32, tag="oev", name=f"oev{h}_{b}_{p}")
                nc.vector.tensor_copy(oev[:], pv[:])
                for t in range(4):
                    nc.tensor.transpose(
                        otr[:, t, :],
                        oev[:, P * t : P * (t + 1)],
                        identity[0 : D + 1, 0 : D + 1],
                    )
                osb = wpool.tile([P, 4, D + 1], F32, tag="osb", name=f"osb{h}_{b}_{p}")
                nc.vector.tensor_copy(osb[:], otr[:])
                oout = wpool.tile([P, 4, D], F32, tag="oout", name=f"oout{h}_{b}_{p}")
                for t in range(4):
                    nc.gpsimd.normalize_recip(
                        oout[:, t, :], osb[:, t, 0:D], osb[:, t, D : D + 1]
                    )
                nc.sync.dma_start(
                    o[h].rearrange("(p c) d -> p c d", p=P)[
                        :, 8 * b + p : 8 * b + 8 : 2, :
                    ],
                    oout[:],
                )

            def emit_pv_steps(u_prev, s):
                h, b = units[u_prev]
                p = s // 8
                pT = pT_of[u_prev]
                if s % 8 == 0:
                    pv_of[u_prev * 2 + p] = pvpool.tile(
                        [D + 1, QB], F32, tag="pv", name=f"pv{h}_{b}_{p}",
                        padded_shape=[P, QB],
                    )
                pv = pv_of[u_prev * 2 + p]
                for c in (2 * (s % 8), 2 * (s % 8) + 1):
                    tt, jj, sl = pt_idx(p, c)
                    nc.tensor.matmul(
                        pv[:],
                        lhsT=vp[:, c, :],
                        rhs=pT[:, tt, jj, sl, :],
                        start=(c == 0),
                        stop=(c == NJ - 1),
                        skip_group_check=True,
                    )
                if s % 8 == 7:
                    emit_output(h, b, p, pv)

            for u, (h, b) in enumerate(units):
                if b == 0:
                    qTs = qTs_of[h]
                pT = ptpool.tile([P, 2, NG, 2, QB], BF16, tag="pT", name=f"pT{h}_{b}")
                pT_of[u] = pT
                for s in range(16):
                    jj, tt = s % 8, s // 8
                    kt_src = kTs if tt == 0 else kTs2
                    sg = sgpool.tile([P, 2, QB], F32, tag="sg", name=f"sg{h}_{b}_{s}")
                    nc.tensor.matmul(
                        sg[:, 0, :],
                        lhsT=kt_src[0:64, jj, :],
                        rhs=qTs[0:64, 4 * b : 4 * (b + 1), :],
                        start=True, stop=True,
                    )
                    nc.tensor.matmul(
                        sg[:, 1, :],
                        lhsT=kt_src[64:P, jj, :],
                        rhs=qTs[64:P, 4 * b : 4 * (b + 1), :],
                        start=True, stop=True,
                    )
                    if u > 0:
                        emit_pv_steps(u - 1, s)
                    out_ap = pT[:, tt, jj, :, :]
                    if (tt, jj) in dve_tiles:
                        nc.vector.tensor_scalar(
                            out_ap.bitcast(I16),
                            sg[:],
                            A16,
                            B16,
                            mybir.AluOpType.mult,
                            mybir.AluOpType.add,
                        )
                    else:
                        nc.scalar.activation(
                            out_ap,
                            sg[:],
                            mybir.ActivationFunctionType.Exp,
                            scale=SCALE,
                        )
                    if s == 3 and b == 1 and h + 1 < HPC:
                        qTs_of[h + 1] = load_qT(h + 1)

            for s in range(16):
                emit_pv_steps(len(units) - 1, s)


def _get_module(**cfg):
    key = tuple(sorted(cfg.items()))
    if key not in _CACHED:
        _CACHED[key] = _build_module(**cfg)
    return _CACHED[key]


def make_in_maps(Q, K, V):
    Q = np.asarray(Q, dtype=np.float32)
    K = np.asarray(K, dtype=np.float32)
    V = np.asarray(V, dtype=np.float32)
    in_maps = []
    for c in range(N_CORES):
        b = c // (N_CORES // B)
        h0 = HPC * (c % (N_CORES // B))
        in_maps.append(
            {
                "q": np.ascontiguousarray(Q[b, h0 : h0 + HPC]),
                "k": np.ascontiguousarray(K[b, 0]),
                "v": np.ascontiguousarray(V[b, 0]),
            }
        )
    return in_maps


def assemble_output(results):
    out = np.empty((B, H, S, D), dtype=np.float32)
    for c in range(N_CORES):
        b = c // (N_CORES // B)
        h0 = HPC * (c % (N_CORES // B))
        out[b, h0 : h0 + HPC] = results[c]["o"]
    return out


def kernel(Q, K, V):
    nc = _get_module(**DEFAULT_CFG)
    res = run_bass_kernel_spmd(nc, make_in_maps(Q, K, V), core_ids=list(range(N_CORES)))
    return assemble_output(res.results)


# revision 5
# speedup vs baseline: 1.2886x; 1.0118x over previous
"""MQA attention kernel v3 for Trainium2 (8 NeuronCores, Bass/Tile).

Problem: Q [2,16,2048,64], K/V [2,1,2048,64] fp32, out = softmax(QK^T/8) V.
Sharding: 32 (batch, head) pairs over 8 cores -> 4 heads/core; one batch's
K/V per core.

v3 = v2's data path with a software-pipelined emission order:
  - bf16 everywhere; Q^T/K^T via gpsimd DMA-cast + xbar DMA transposes of
    [128,128] chunk-pairs (zero PE/DVE cost). kTs + half-swapped kTs2 give
    every chunk in both partition halves.
  - Unit = (head, 512-col q-block): 16 score steps per unit; each step is
    one [64-deep] QK matmul pair (parities in the two PE row halves), one
    exp instruction (ScalarE table-exp for 12 steps, VectorE Schraudolph
    int16->bf16 for 4), plus TWO PV-chain matmuls of the PREVIOUS unit
    threaded between - so ScalarE never idles at unit boundaries.
  - PV: full-128-deep accumulation chains (one PSUM bank per col-group,
    LDWEIGHTS hides in the background weight buffer).
  - Output: PE transpose -> DVE PSUM->SBUF copy -> gpsimd normalize_recip
    (the only engine with slack) -> DMA store.

The q columns are processed in a (parity, block) interleaved order; the
output store APs undo the permutation (col-group (b, parity p, sub t)
holds global q with q mod 16 == 8b + 2t + p).
"""

import numpy as np

import concourse.bass as bass
import concourse.mybir as mybir
import concourse.tile as tile
from concourse import bacc
from concourse.bass_utils import run_bass_kernel_spmd
from concourse.masks import make_identity

B, H, S, D = 2, 16, 2048, 64
N_CORES = 8
HPC = (B * H) // N_CORES
P = 128
NJ = S // P
NG = NJ // 2
QB = 512
LAG = 4
SCALE = 1.0 / float(D) ** 0.5
F32 = mybir.dt.float32
BF16 = mybir.dt.bfloat16
I16 = mybir.dt.int16

LOG2E = 1.4426950408889634
A16 = float((1 << 7) * LOG2E * SCALE)
B16 = float(127.0 * (1 << 7) - 7.42)

# exp steps (tt, jj) run on the DVE via Schraudolph; listing both tt of a
# jj approximates chunks {2jj, 2jj+1} fully.
DVE_TILES = ((0, 1), (0, 3), (0, 5), (1, 1), (1, 3))

_CACHED = {}
DEFAULT_CFG = {}


def _build_module(**cfg):
    nc = bacc.Bacc(None)
    q = nc.dram_tensor("q", [HPC, S, D], F32, kind="ExternalInput")
    k = nc.dram_tensor("k", [S, D], F32, kind="ExternalInput")
    v = nc.dram_tensor("v", [S, D], F32, kind="ExternalInput")
    o = nc.dram_tensor("o", [HPC, S, D], F32, kind="ExternalOutput")
    _trace_body(nc, q, k, v, o, **cfg)
    nc.compile()
    return nc


def _trace_body(nc, q, k, v, o, sg_bufs=3, pv_bufs=2, pt_bufs=3, dve_tiles=None):
    dve_tiles = DVE_TILES if dve_tiles is None else dve_tiles
    with tile.TileContext(nc) as tc:
        with (
            tc.tile_pool(name="const", bufs=1) as cpool,
            tc.tile_pool(name="qb", bufs=2) as qpool,
            tc.tile_pool(name="ptb", bufs=pt_bufs) as ptpool,
            tc.tile_pool(name="wk", bufs=2) as wpool,
            tc.tile_pool(name="sg", bufs=sg_bufs, space="PSUM") as sgpool,
            tc.tile_pool(name="pv", bufs=pv_bufs, space="PSUM") as pvpool,
        ):
            k_bf = cpool.tile([P, NJ, D], BF16)
            k_src = k.rearrange("(p c) d -> p c d", p=P)
            nc.gpsimd.dma_start(k_bf[:, 0 : NJ // 2, :], k_src[:, 0 : NJ // 2, :])
            nc.gpsimd.dma_start(k_bf[:, NJ // 2 :, :], k_src[:, NJ // 2 :, :])
            kTs = cpool.tile([P, NG, P], BF16)
            kTs2 = cpool.tile([P, NG, P], BF16)
            for g in range(NG):
                nc.scalar.dma_start_transpose(
                    kTs[:, g, :], k_bf[:, 2 * g : 2 * g + 2, :]
                )
            nc.sync.dma_start(kTs2[0:64, :, :], kTs[64:P, :, :])
            nc.sync.dma_start(kTs2[64:P, :, :], kTs[0:64, :, :])

            identity = cpool.tile([P, P], F32)
            make_identity(nc, identity)
            identity_bf = cpool.tile([P, P], BF16)
            nc.vector.tensor_copy(identity_bf[:], identity[:])

            v_nat = cpool.tile([P, NJ, D], F32)
            nc.sync.dma_start(v_nat[:], v.rearrange("(p c) d -> p c d", p=P))
            vp = cpool.tile([P, NJ, D + 1], BF16)
            nc.gpsimd.memset(vp[:, :, D], 1.0)
            nc.vector.tensor_copy(vp[:, :, 0:D], v_nat[:])

            def load_qT(h):
                q_bf = qpool.tile([P, NJ, D], BF16, tag="qbf", name=f"qbf{h}")
                q_src = q[h].rearrange("(p c) d -> p c d", p=P)
                nc.gpsimd.dma_start(q_bf[:, 0 : NJ // 2, :], q_src[:, 0 : NJ // 2, :])
                nc.gpsimd.dma_start(q_bf[:, NJ // 2 :, :], q_src[:, NJ // 2 :, :])
                qTs = qpool.tile([P, NG, P], BF16, tag="qTs", name=f"qTs{h}")
                for g in range(NG):
                    nc.sync.dma_start_transpose(
                        qTs[:, g, :], q_bf[:, 2 * g : 2 * g + 2, :]
                    )
                return qTs

            def pt_idx(p, c):
                jj, jp = divmod(c, 2)
                if p == 0:
                    return (0, jj, 0) if jp == 0 else (1, jj, 0)
                return (0, jj, 1) if jp == 1 else (1, jj, 1)

            qTs_cur = load_qT(0)

            # ---- steady-state pipeline over units u = (h, b); PV of unit
            # u-1 threads through unit u's score steps ----
            units = [(h, b) for h in range(HPC) for b in range(2)]
            qTs_of = {0: qTs_cur}
            pT_of = {}
            pv_of = {}

            def pt_idx(p, c):
                jj, jp = divmod(c, 2)
                if p == 0:
                    return (0, jj, 0) if jp == 0 else (1, jj, 0)
                return (0, jj, 1) if jp == 1 else (1, jj, 1)

            def emit_output(h, b, p, pv):
                otr = pvpool.tile(
                    [P, 4, D + 1], BF16, tag="pv", name=f"otr{h}_{b}_{p}",
                    padded_shape=[P, 4, QB // 2],
                )
                oev = wpool.tile([D + 1, QB], BF16, tag="oev", name=f"oev{h}_{b}_{p}")
                nc.vector.tensor_copy(oev[:], pv[:])
                for t in range(4):
                    nc.tensor.transpose(
                        otr[:, t, :],
                        oev[:, P * t : P * (t + 1)],
                        identity_bf[0 : D + 1, 0 : D + 1],
                    )
                osb = wpool.tile([P, 4, D + 1], F32, tag="osb", name=f"osb{h}_{b}_{p}")
                nc.vector.tensor_copy(osb[:], otr[:])
                oout = wpool.tile([P, 4, D], F32, tag="oout", name=f"oout{h}_{b}_{p}")
                for t in range(4):
                    nc.gpsimd.normalize_recip(
                        oout[:, t, :], osb[:, t, 0:D], osb[:, t, D : D + 1]
                    )
                nc.sync.dma_start(
                    o[h].rearrange("(p c) d -> p c d", p=P)[
                        :, 8 * b + p : 8 * b + 8 : 2, :
                    ],
                    oout[:],
                )

            def emit_pv_steps(u_prev, s):
                h, b = units[u_prev]
                p = s // 8
                pT = pT_of[u_prev]
                if s % 8 == 0:
                    pv_of[u_prev * 2 + p] = pvpool.tile(
                        [D + 1, QB], F32, tag="pv", name=f"pv{h}_{b}_{p}",
                        padded_shape=[P, QB],
                    )
                pv = pv_of[u_prev * 2 + p]
                for c in (2 * (s % 8), 2 * (s % 8) + 1):
                    tt, jj, sl = pt_idx(p, c)
                    nc.tensor.matmul(
                        pv[:],
                        lhsT=vp[:, c, :],
                        rhs=pT[:, tt, jj, sl, :],
                        start=(c == 0),
                        stop=(c == NJ - 1),
                        skip_group_check=True,
                    )
                if s % 8 == 7:
                    emit_output(h, b, p, pv)

            for u, (h, b) in enumerate(units):
                if b == 0:
                    qTs = qTs_of[h]
                pT = ptpool.tile([P, 2, NG, 2, QB], BF16, tag="pT", name=f"pT{h}_{b}")
                pT_of[u] = pT
                for s in range(16):
                    jj, tt = s % 8, s // 8
                    kt_src = kTs if tt == 0 else kTs2
                    sg = sgpool.tile([P, 2, QB], F32, tag="sg", name=f"sg{h}_{b}_{s}")
                    nc.tensor.matmul(
                        sg[:, 0, :],
                        lhsT=kt_src[0:64, jj, :],
                        rhs=qTs[0:64, 4 * b : 4 * (b + 1), :],
                        start=True, stop=True,
                    )
                    nc.tensor.matmul(
                        sg[:, 1, :],
                        lhsT=kt_src[64:P, jj, :],
                        rhs=qTs[64:P, 4 * b : 4 * (b + 1), :],
                        start=True, stop=True,
                    )
                    if u > 0:
                        emit_pv_steps(u - 1, s)
                    out_ap = pT[:, tt, jj, :, :]
                    if (tt, jj) in dve_tiles:
                        nc.vector.tensor_scalar(
                            out_ap.bitcast(I16),
                            sg[:],
                            A16,
                            B16,
                            mybir.AluOpType.mult,
                            mybir.AluOpType.add,
                        )
                    else:
                        nc.scalar.activation(
                            out_ap,
                            sg[:],
                            mybir.ActivationFunctionType.Exp,
                            scale=SCALE,
                        )
                    if s == 3 and b == 1 and h + 1 < HPC:
                        qTs_of[h + 1] = load_qT(h + 1)

            for s in range(16):
                emit_pv_steps(len(units) - 1, s)


def _get_module(**cfg):
    key = tuple(sorted(cfg.items()))
    if key not in _CACHED:
        _CACHED[key] = _build_module(**cfg)
    return _CACHED[key]


def make_in_maps(Q, K, V):
    Q = np.asarray(Q, dtype=np.float32)
    K = np.asarray(K, dtype=np.float32)
    V = np.asarray(V, dtype=np.float32)
    in_maps = []
    for c in range(N_CORES):
        b = c // (N_CORES // B)
        h0 = HPC * (c % (N_CORES // B))
        in_maps.append(
            {
                "q": np.ascontiguousarray(Q[b, h0 : h0 + HPC]),
                "k": np.ascontiguousarray(K[b, 0]),
                "v": np.ascontiguousarray(V[b, 0]),
            }
        )
    return in_maps


def assemble_output(results):
    out = np.empty((B, H, S, D), dtype=np.float32)
    for c in range(N_CORES):
        b = c // (N_CORES // B)
        h0 = HPC * (c % (N_CORES // B))
        out[b, h0 : h0 + HPC] = results[c]["o"]
    return out


def kernel(Q, K, V):
    nc = _get_module(**DEFAULT_CFG)
    res = run_bass_kernel_spmd(nc, make_in_maps(Q, K, V), core_ids=list(range(N_CORES)))
    return assemble_output(res.results)
